# revision 1
# baseline (speedup 1.0000x reference)
"""CorrelationLayer1D Trainium2 Bass kernel.

Computes out[b, d, h, w] = sum_c x_1[b,c,h,w] * x2p[b,c,h,w+d] for d in [0, 41),
where x2p is x_2 width-padded by (8, 32).  Inputs [4,128,160,320] f32.

Sharding: data-parallel over H = 160 = 8*20 (correlation runs along W only, so
H-sharding needs no halo).  Per core, rows are processed in chunks of HC=10:

  per (b, h) row, per w-block (128/128/64):
    PE computes a block-diagonal Gram: two M=64 matmuls (lo: cols [w0,w0+64) x
    window [w0,w0+104); hi: cols [w0+64,w0+128) x window [w0+64,w0+168)) land
    as one compacted [128,104] PSUM tile - the 41-wide correlation band of
    partition i lives at free offset (i mod 64)+d.
  ScalarE copies each Gram into a per-chunk SBUF atlas [128, HC*312].
  Per chunk, 5 DMAs write the atlas to DRAM scratch and 5 skewed reloads
  (flat DRAM-side APs, step 105 = row-pitch+1) extract the band as
  S[i, (h,d)] - DRAM APs allow arbitrary affine steps; SBUF-side per-partition
  skews are not expressible, which is why the round-trip exists.
  PE transposes each row's S [wb,41] -> T[d, i]; VectorE copies T into a
  per-chunk assembly tile [41, HC*320]; one strided DMA per chunk stores it.
"""

import sys

import numpy as np

try:
    import concourse.bass as bass  # noqa: F401
except ImportError:
    sys.path.insert(0, "/opt/trn_rl_repo")

import concourse.bass as bass
import concourse.tile as tile
from concourse import bacc, masks, mybir
from concourse.ap import AP
from concourse.bass_utils import run_bass_kernel_spmd

MAX_DISP = 40
D = MAX_DISP + 1  # 41 displacements
PAD_L = 8
PAD_R = 32
B, C, H, W = 4, 128, 160, 320
N_CORES = 8
HS = H // N_CORES  # 20 h-rows per core
WP = W + PAD_L + PAD_R  # 360
WBLOCKS = [(0, 128), (128, 128), (256, 64)]
GW = 104  # compacted gram width per block: 64 + MAX_DISP
AW = 3 * GW  # atlas width per row: 312

F32 = mybir.dt.float32


def build_kernel(b_dim=B, hs=HS, hc=None):
    if hc is None:
        hc = 10 if hs % 10 == 0 else hs
    assert hs % hc == 0
    nchunks = hs // hc

    nc = bacc.Bacc("TRN2", target_bir_lowering=False, debug=False)
    x1e = nc.declare_dram_parameter("x1", [b_dim, C, hs, W], F32, isOutput=False)
    x2e = nc.declare_dram_parameter("x2", [b_dim, C, hs, W], F32, isOutput=False)
    oute = nc.declare_dram_parameter("out", [b_dim, D, hs, W], F32, isOutput=True)

    with tile.TileContext(nc) as tc:
        with (
            tc.tile_pool(name="const", bufs=1) as const_pool,
            tc.tile_pool(name="xin", bufs=3) as xin_pool,
            tc.tile_pool(name="atlas", bufs=2) as atlas_pool,
            tc.tile_pool(name="sbig", bufs=2) as sbig_pool,
            tc.tile_pool(name="asm", bufs=2) as asm_pool,
            tc.tile_pool(name="psum_g", bufs=4, space="PSUM") as psum_g,
            tc.tile_pool(name="psum_t", bufs=3, space="PSUM") as psum_t,
            tc.tile_pool(name="scratch", bufs=6, space="DRAM") as scratch_pool,
        ):
            identity = const_pool.tile([128, 128], F32)
            masks.make_identity(nc, identity[:])

            for b in range(b_dim):
                for ci in range(nchunks):
                    h0 = ci * hc
                    x1b = xin_pool.tile([C, hc * W], F32, tag="x1b")
                    nc.sync.dma_start(
                        x1b[:].rearrange("p (h w) -> p h w", w=W),
                        x1e[b, :, h0 : h0 + hc, :],
                    )
                    x2b = xin_pool.tile([C, hc * WP], F32, tag="x2b")
                    x2b3 = x2b[:].rearrange("p (h w) -> p h w", w=WP)
                    nc.gpsimd.memset(x2b3[:, :, 0:PAD_L], 0.0)
                    nc.gpsimd.memset(x2b3[:, :, PAD_L + W : WP], 0.0)
                    nc.sync.dma_start(
                        x2b3[:, :, PAD_L : PAD_L + W], x2e[b, :, h0 : h0 + hc, :]
                    )

                    atlas = atlas_pool.tile([C, hc * AW], F32, tag="atlas")
                    abatch = asm_pool.tile([D, hc * W], F32, tag="abatch")
                    sbig = [
                        sbig_pool.tile([128, hc * D], F32, tag="sbig0", name=f"sbig0_{b}_{ci}"),
                        sbig_pool.tile([128, hc * D], F32, tag="sbig1", name=f"sbig1_{b}_{ci}"),
                        sbig_pool.tile([64, hc * D], F32, tag="sbig2", name=f"sbig2_{b}_{ci}"),
                    ]

                    # Gram compute + PSUM->SBUF atlas copies, row by row.
                    for hh in range(hc):
                        o1 = hh * W
                        o2 = hh * WP
                        for kblk, (w0, wb) in enumerate(WBLOCKS):
                            gram_ps = psum_g.tile([wb, GW], F32, tag="gram")
                            nc.tensor.matmul(
                                gram_ps[0:64, :],
                                x1b[:, o1 + w0 : o1 + w0 + 64],
                                x2b[:, o2 + w0 : o2 + w0 + GW],
                                start=True,
                                stop=True,
                                tile_position=(0, 0),
                            )
                            if wb == 128:
                                nc.tensor.matmul(
                                    gram_ps[64:128, :],
                                    x1b[:, o1 + w0 + 64 : o1 + w0 + 128],
                                    x2b[:, o2 + w0 + 64 : o2 + w0 + 64 + GW],
                                    start=True,
                                    stop=True,
                                    tile_position=(0, 64),
                                )
                            aoff = kblk * hc * GW + hh * GW
                            nc.scalar.copy(
                                atlas[0:wb, aoff : aoff + GW],
                                gram_ps[:],
                            )

                    # Scratch round-trip: rectangular store, skewed reload.
                    for kblk, half in [(0, 0), (0, 1), (1, 0), (1, 1), (2, 0)]:
                        scr = scratch_pool.tile([64, hc * GW], F32, tag="scr")
                        nc.sync.dma_start(
                            scr[:],
                            atlas[
                                half * 64 : (half + 1) * 64,
                                kblk * hc * GW : (kblk + 1) * hc * GW,
                            ],
                        )

                        scr_ap = scr[:]
                        diag = AP(
                            tensor=scr_ap.tensor,
                            offset=scr_ap.offset,
                            ap=[[hc * GW + 1, 64], [GW, hc], [1, D]],
                        )
                        sb = sbig[kblk]
                        dstp = sb[half * 64 : (half + 1) * 64, :].rearrange(
                            "p (h d) -> p h d", d=D
                        )
                        nc.scalar.dma_start(dstp, diag)

                    # Transpose each row-block band to [d, w] and assemble.
                    for hh in range(hc):
                        for kblk, (w0, wb) in enumerate(WBLOCKS):
                            sb = sbig[kblk]
                            t_ps = psum_t.tile([D, wb], F32, tag="t_ps")
                            nc.tensor.matmul(
                                t_ps[:],
                                sb[0:wb, hh * D : (hh + 1) * D],
                                identity[0:wb, 0:wb],
                                start=True,
                                stop=True,
                                is_transpose=True,
                            )
                            nc.vector.tensor_copy(
                                abatch[:, hh * W + w0 : hh * W + w0 + wb], t_ps[:]
                            )

                    nc.scalar.dma_start(
                        oute[b, :, h0 : h0 + hc, :],
                        abatch[:].rearrange("d (h w) -> d h w", w=W),
                    )

    nc.finalize()
    return nc


_compiled = {}


def _get_kernel(b_dim, hs):
    key = (b_dim, hs)
    if key not in _compiled:
        _compiled[key] = build_kernel(b_dim, hs)
    return _compiled[key]


def kernel(x_1: np.ndarray, x_2: np.ndarray) -> np.ndarray:
    assert x_1.shape == (B, C, H, W) and x_2.shape == (B, C, H, W)
    x_1 = np.ascontiguousarray(x_1, dtype=np.float32)
    x_2 = np.ascontiguousarray(x_2, dtype=np.float32)
    nc = _get_kernel(B, HS)
    in_maps = [
        {
            "x1": np.ascontiguousarray(x_1[:, :, i * HS : (i + 1) * HS, :]),
            "x2": np.ascontiguousarray(x_2[:, :, i * HS : (i + 1) * HS, :]),
        }
        for i in range(N_CORES)
    ]
    res = run_bass_kernel_spmd(nc, in_maps, core_ids=list(range(N_CORES))).results
    out = np.concatenate([res[i]["out"] for i in range(N_CORES)], axis=2)
    return out



# revision 6
# speedup vs baseline: 1.4798x; 1.4798x over previous
"""CorrelationLayer1D Trainium2 Bass kernel (v2).

Computes out[b, d, h, w] = sum_c x_1[b,c,h,w] * x2p[b,c,h,w+d] for d in [0, 41),
where x2p is x_2 width-padded by (8, 32).  Inputs [4,128,160,320] f32.

Sharding: data-parallel over H = 160 = 8*20 (correlation runs along W only, so
H-sharding needs no halo).  Per core, rows are processed in chunks of HC=10.

v2 changes vs v1 (443us):
 - inputs cast f32->bf16 during the load DMA (SWDGE/gpsimd cast path); Gram
   matmuls run in bf16 at 1 cyc/row instead of fp32's 4.
 - Gram blocks shrink to M=32 stationary columns, 4-way col-tiled via
   tile_position (0,0)/(0,32)/(0,64)/(0,96) into one [128,72] PSUM tile.
   Atlas per row is 10*32*72 elems vs 5*64*104 - 31% fewer, and it is stored
   to DRAM scratch in bf16, halving bytes again.
 - band extraction (the per-partition skew S[i,d] = G[i, i+d], inexpressible
   in SBUF-side APs) still round-trips through DRAM scratch, but with one
   4D skewed AP per 128-partition group, issued on gpsimd (SWDGE: 994ns +
   0.34ns/desc) instead of HWDGE which blocked ScalarE ~2.2us per issue.
 - engine assignment: gpsimd = input-cast loads + band reads + memsets,
   sync = scratch writes + output stores, scalar = PSUM->atlas copies (+ the
   64-wide transpose evacuations), vector = 128-wide transpose evacuations,
   tensor = Grams + per-row band transposes (bf16).

Per (b, h-row) per 32-col block k (w0 = 32k): PE computes G[i, j] =
sum_c x1[c, w0+i] * x2p[c, w0+32*(i//32)... packed so partition p of the
[128,72] tile holds w-col 128*kgrp+p with band at j = (p%32)+d.  ScalarE
copies the tile (cast to bf16) into a per-chunk atlas [wb, HC*72]; one DMA
stores it to scratch; one skewed 4D reload extracts S[p, (h,d)] (41-elem
contiguous runs); PE transposes each row's S [wb, 41] -> [41, wb]; DVE/ACT
copy into abatch [41, HC*320]; one strided DMA per chunk stores d-major.
"""

import sys

import numpy as np

try:
    import concourse.bass as bass  # noqa: F401
except ImportError:
    sys.path.insert(0, "/opt/trn_rl_repo")

import concourse.bass as bass
import concourse.tile as tile
from concourse import bacc, masks, mybir
from concourse.ap import AP
from concourse.bass_utils import run_bass_kernel_spmd

MAX_DISP = 40
D = MAX_DISP + 1  # 41 displacements
PAD_L = 8
PAD_R = 32
B, C, H, W = 4, 128, 160, 320
N_CORES = 8
HS = H // N_CORES  # 20 h-rows per core
WP = W + PAD_L + PAD_R  # 360
WGRPS = [128, 128, 64]  # partition-group widths covering W=320
GW = 32 + MAX_DISP  # compacted gram width per 32-col block: 72

F32 = mybir.dt.float32
BF16 = mybir.dt.bfloat16


def build_kernel(b_dim=B, hs=HS, hc=None):
    if hc is None:
        hc = 10 if hs % 10 == 0 else hs
    assert hs % hc == 0
    nchunks = hs // hc

    nc = bacc.Bacc("TRN2", target_bir_lowering=False, debug=False)
    x1e = nc.declare_dram_parameter("x1", [b_dim, C, hs, W], F32, isOutput=False)
    x2e = nc.declare_dram_parameter("x2", [b_dim, C, hs, W], F32, isOutput=False)
    oute = nc.declare_dram_parameter("out", [b_dim, D, hs, W], F32, isOutput=True)

    with tile.TileContext(nc) as tc:
        with (
            tc.tile_pool(name="const", bufs=1) as const_pool,
            tc.tile_pool(name="xin", bufs=3) as xin_pool,
            tc.tile_pool(name="atlas", bufs=2) as atlas_pool,
            tc.tile_pool(name="sbig", bufs=2) as sbig_pool,
            tc.tile_pool(name="asm", bufs=2) as asm_pool,
            tc.tile_pool(name="psum_g", bufs=4, space="PSUM") as psum_g,
            tc.tile_pool(name="psum_t", bufs=3, space="PSUM") as psum_t,
            tc.tile_pool(name="scratch", bufs=6, space="DRAM") as scratch_pool,
        ):
            identity = const_pool.tile([128, 128], BF16)
            masks.make_identity(nc, identity[:])

            for b in range(b_dim):
                for ci in range(nchunks):
                    h0 = ci * hc
                    x1b = xin_pool.tile([C, hc * W], BF16, tag="x1b")
                    nc.gpsimd.dma_start(
                        x1b[:].rearrange("p (h w) -> p h w", w=W),
                        x1e[b, :, h0 : h0 + hc, :],
                    )
                    x2b = xin_pool.tile([C, hc * WP], BF16, tag="x2b")
                    x2b3 = x2b[:].rearrange("p (h w) -> p h w", w=WP)
                    nc.gpsimd.memset(x2b3[:, :, 0:PAD_L], 0.0)
                    nc.gpsimd.memset(x2b3[:, :, PAD_L + W : WP], 0.0)
                    nc.gpsimd.dma_start(
                        x2b3[:, :, PAD_L : PAD_L + W], x2e[b, :, h0 : h0 + hc, :]
                    )

                    atlas = [
                        atlas_pool.tile(
                            [wb, hc * GW], BF16, tag=f"atl{k}", name=f"atl{k}_{b}_{ci}"
                        )
                        for k, wb in enumerate(WGRPS)
                    ]

                    # Gram compute + PSUM->SBUF atlas copies (cast to bf16).
                    for hh in range(hc):
                        o1 = hh * W
                        o2 = hh * WP
                        for kgrp, wb in enumerate(WGRPS):
                            gram_ps = psum_g.tile(
                                [wb, GW], F32, tag="gram", name=f"gram_{b}_{ci}_{hh}_{kgrp}"
                            )
                            for kk in range(wb // 32):
                                w0 = 128 * kgrp + 32 * kk
                                nc.tensor.matmul(
                                    gram_ps[32 * kk : 32 * kk + 32, :],
                                    x1b[:, o1 + w0 : o1 + w0 + 32],
                                    x2b[:, o2 + w0 : o2 + w0 + GW],
                                    start=True,
                                    stop=True,
                                    tile_position=(0, 32 * kk),
                                )
                            nc.scalar.copy(
                                atlas[kgrp][:, hh * GW : (hh + 1) * GW], gram_ps[:]
                            )

                    # Scratch round-trip.  The write deskews: block bb's rows land
                    # at addr = 32*ALPHA*bb + hc*GW*i + GW*h + j, so the band
                    # S[p,h,d] = G[p,h,(p%32)+d] sits at the UNIFORM affine
                    # address ALPHA*p + GW*h + d and reloads as one plain 3D AP.
                    ALPHA = hc * GW + 1
                    sbig = []
                    for kgrp, wb in enumerate(WGRPS):
                        scr = scratch_pool.tile(
                            [wb * ALPHA], BF16, tag=f"scr{kgrp}", name=f"scr{kgrp}_{b}_{ci}"
                        )
                        scr_ap = scr[:]
                        for bb in range(wb // 32):
                            dst = AP(
                                tensor=scr_ap.tensor,
                                offset=scr_ap.offset + 32 * ALPHA * bb,
                                ap=[[hc * GW, 32], [GW, hc], [1, GW]],
                            )
                            nc.sync.dma_start(
                                dst, atlas[kgrp][32 * bb : 32 * bb + 32, :]
                            ) if bb % 2 == 0 else nc.scalar.dma_start(
                                dst, atlas[kgrp][32 * bb : 32 * bb + 32, :]
                            )

                        sb = sbig_pool.tile(
                            [wb, hc * D], BF16, tag=f"sb{kgrp}", name=f"sb{kgrp}_{b}_{ci}"
                        )
                        band = AP(
                            tensor=scr_ap.tensor,
                            offset=scr_ap.offset,
                            ap=[[ALPHA, wb], [GW, hc], [1, D]],
                        )
                        nc.gpsimd.dma_start(
                            sb[:].rearrange("p (h d) -> p h d", d=D), band
                        )
                        sbig.append(sb)

                    # Transpose each row-block band to [d, w] and assemble.
                    abatch = asm_pool.tile([D, hc * W], F32, tag="abatch")
                    for hh in range(hc):
                        for kgrp, wb in enumerate(WGRPS):
                            t_ps = psum_t.tile([D, wb], BF16, tag="t_ps")
                            nc.tensor.matmul(
                                t_ps[:],
                                sbig[kgrp][:, hh * D : (hh + 1) * D],
                                identity[0:wb, 0:wb],
                                start=True,
                                stop=True,
                                is_transpose=True,
                            )
                            dst = abatch[:, hh * W + 128 * kgrp : hh * W + 128 * kgrp + wb]
                            if kgrp < 2:
                                nc.vector.tensor_copy(dst, t_ps[:])
                            else:
                                nc.scalar.copy(dst, t_ps[:])

                    nc.sync.dma_start(
                        oute[b, :, h0 : h0 + hc, :],
                        abatch[:].rearrange("d (h w) -> d h w", w=W),
                    )

    nc.finalize()
    return nc


_compiled = {}


def _get_kernel(b_dim, hs):
    key = (b_dim, hs)
    if key not in _compiled:
        _compiled[key] = build_kernel(b_dim, hs)
    return _compiled[key]


def kernel(x_1: np.ndarray, x_2: np.ndarray) -> np.ndarray:
    assert x_1.shape == (B, C, H, W) and x_2.shape == (B, C, H, W)
    x_1 = np.ascontiguousarray(x_1, dtype=np.float32)
    x_2 = np.ascontiguousarray(x_2, dtype=np.float32)
    nc = _get_kernel(B, HS)
    in_maps = [
        {
            "x1": np.ascontiguousarray(x_1[:, :, i * HS : (i + 1) * HS, :]),
            "x2": np.ascontiguousarray(x_2[:, :, i * HS : (i + 1) * HS, :]),
        }
        for i in range(N_CORES)
    ]
    res = run_bass_kernel_spmd(nc, in_maps, core_ids=list(range(N_CORES))).results
    out = np.concatenate([res[i]["out"] for i in range(N_CORES)], axis=2)
    return out


# revision 9
# speedup vs baseline: 1.5905x; 1.0748x over previous
"""CorrelationLayer1D Trainium2 Bass kernel (v2).

Computes out[b, d, h, w] = sum_c x_1[b,c,h,w] * x2p[b,c,h,w+d] for d in [0, 41),
where x2p is x_2 width-padded by (8, 32).  Inputs [4,128,160,320] f32.

Sharding: data-parallel over H = 160 = 8*20 (correlation runs along W only, so
H-sharding needs no halo).  Per core, rows are processed in chunks of HC=10.

v2 changes vs v1 (443us):
 - inputs cast f32->bf16 during the load DMA (SWDGE/gpsimd cast path); Gram
   matmuls run in bf16 at 1 cyc/row instead of fp32's 4.
 - Gram blocks shrink to M=32 stationary columns, 4-way col-tiled via
   tile_position (0,0)/(0,32)/(0,64)/(0,96) into one [128,72] PSUM tile.
   Atlas per row is 10*32*72 elems vs 5*64*104 - 31% fewer, and it is stored
   to DRAM scratch in bf16, halving bytes again.
 - band extraction (the per-partition skew S[i,d] = G[i, i+d], inexpressible
   in SBUF-side APs) still round-trips through DRAM scratch, but with one
   4D skewed AP per 128-partition group, issued on gpsimd (SWDGE: 994ns +
   0.34ns/desc) instead of HWDGE which blocked ScalarE ~2.2us per issue.
 - engine assignment: gpsimd = input-cast loads + band reads + memsets,
   sync = scratch writes + output stores, scalar = PSUM->atlas copies (+ the
   64-wide transpose evacuations), vector = 128-wide transpose evacuations,
   tensor = Grams + per-row band transposes (bf16).

Per (b, h-row) per 32-col block k (w0 = 32k): PE computes G[i, j] =
sum_c x1[c, w0+i] * x2p[c, w0+32*(i//32)... packed so partition p of the
[128,72] tile holds w-col 128*kgrp+p with band at j = (p%32)+d.  ScalarE
copies the tile (cast to bf16) into a per-chunk atlas [wb, HC*72]; one DMA
stores it to scratch; one skewed 4D reload extracts S[p, (h,d)] (41-elem
contiguous runs); PE transposes each row's S [wb, 41] -> [41, wb]; DVE/ACT
copy into abatch [41, HC*320]; one strided DMA per chunk stores d-major.
"""

import sys

import numpy as np

try:
    import concourse.bass as bass  # noqa: F401
except ImportError:
    sys.path.insert(0, "/opt/trn_rl_repo")

import concourse.bass as bass
import concourse.tile as tile
from concourse import bacc, masks, mybir
from concourse.ap import AP
from concourse.bass_utils import run_bass_kernel_spmd

MAX_DISP = 40
D = MAX_DISP + 1  # 41 displacements
PAD_L = 8
PAD_R = 32
B, C, H, W = 4, 128, 160, 320
N_CORES = 8
HS = H // N_CORES  # 20 h-rows per core
WP = W + PAD_L + PAD_R  # 360
WGRPS = [128, 128, 64]  # partition-group widths covering W=320
GW = 32 + MAX_DISP  # compacted gram width per 32-col block: 72

F32 = mybir.dt.float32
BF16 = mybir.dt.bfloat16


def build_kernel(b_dim=B, hs=HS, hc=None):
    if hc is None:
        hc = 10 if hs % 10 == 0 else hs
    assert hs % hc == 0
    nchunks = hs // hc

    nc = bacc.Bacc("TRN2", target_bir_lowering=False, debug=False)
    x1e = nc.declare_dram_parameter("x1", [b_dim, C, hs, W], F32, isOutput=False)
    x2e = nc.declare_dram_parameter("x2", [b_dim, C, hs, W], F32, isOutput=False)
    oute = nc.declare_dram_parameter("out", [b_dim, D, hs, W], F32, isOutput=True)

    with tile.TileContext(nc) as tc:
        with (
            tc.tile_pool(name="const", bufs=1) as const_pool,
            tc.tile_pool(name="xin", bufs=3) as xin_pool,
            tc.tile_pool(name="atlas", bufs=2) as atlas_pool,
            tc.tile_pool(name="sbig", bufs=2) as sbig_pool,
            tc.tile_pool(name="asm", bufs=2) as asm_pool,
            tc.tile_pool(name="psum_g", bufs=4, space="PSUM") as psum_g,
            tc.tile_pool(name="psum_t", bufs=3, space="PSUM") as psum_t,
            tc.tile_pool(name="scratch", bufs=6, space="DRAM") as scratch_pool,
        ):
            identity = const_pool.tile([128, 128], BF16)
            masks.make_identity(nc, identity[:])

            for b in range(b_dim):
                for ci in range(nchunks):
                    h0 = ci * hc
                    x1b = xin_pool.tile([C, hc * W], BF16, tag="x1b")
                    nc.gpsimd.dma_start(
                        x1b[:].rearrange("p (h w) -> p h w", w=W),
                        x1e[b, :, h0 : h0 + hc, :],
                    )
                    x2b = xin_pool.tile([C, hc * WP], BF16, tag="x2b")
                    x2b3 = x2b[:].rearrange("p (h w) -> p h w", w=WP)
                    nc.gpsimd.memset(x2b3[:, :, 0:PAD_L], 0.0)
                    nc.gpsimd.memset(x2b3[:, :, PAD_L + W : WP], 0.0)
                    nc.gpsimd.dma_start(
                        x2b3[:, :, PAD_L : PAD_L + W], x2e[b, :, h0 : h0 + hc, :]
                    )

                    atlas = [
                        atlas_pool.tile(
                            [wb, hc * GW], BF16, tag=f"atl{k}", name=f"atl{k}_{b}_{ci}"
                        )
                        for k, wb in enumerate(WGRPS)
                    ]

                    # Gram compute + PSUM->SBUF atlas copies (cast to bf16).
                    for hh in range(hc):
                        o1 = hh * W
                        o2 = hh * WP
                        for kgrp, wb in enumerate(WGRPS):
                            gram_ps = psum_g.tile(
                                [wb, GW], F32, tag="gram", name=f"gram_{b}_{ci}_{hh}_{kgrp}"
                            )
                            for kk in range(wb // 32):
                                w0 = 128 * kgrp + 32 * kk
                                nc.tensor.matmul(
                                    gram_ps[32 * kk : 32 * kk + 32, :],
                                    x1b[:, o1 + w0 : o1 + w0 + 32],
                                    x2b[:, o2 + w0 : o2 + w0 + GW],
                                    start=True,
                                    stop=True,
                                    tile_position=(0, 32 * kk),
                                )
                            nc.scalar.copy(
                                atlas[kgrp][:, hh * GW : (hh + 1) * GW], gram_ps[:]
                            )

                    # Scratch round-trip.  The write deskews: block bb's rows land
                    # at addr = 32*ALPHA*bb + hc*GW*i + GW*h + j, so the band
                    # S[p,h,d] = G[p,h,(p%32)+d] sits at the UNIFORM affine
                    # address ALPHA*p + GW*h + d and reloads as one plain 3D AP.
                    ALPHA = hc * GW + 1
                    sbig = []
                    for kgrp, wb in enumerate(WGRPS):
                        scr = scratch_pool.tile(
                            [wb * ALPHA], BF16, tag=f"scr{kgrp}", name=f"scr{kgrp}_{b}_{ci}"
                        )
                        scr_ap = scr[:]
                        for bb in range(wb // 32):
                            dst = AP(
                                tensor=scr_ap.tensor,
                                offset=scr_ap.offset + 32 * ALPHA * bb,
                                ap=[[hc * GW, 32], [GW, hc], [1, GW]],
                            )
                            nc.sync.dma_start(
                                dst, atlas[kgrp][32 * bb : 32 * bb + 32, :]
                            ) if bb % 2 == 0 else nc.scalar.dma_start(
                                dst, atlas[kgrp][32 * bb : 32 * bb + 32, :]
                            )

                        # Full-plane reload: sb[p, 72h+d] = S[p,h,d]; cols beyond
                        # d=40 in each 72-group are junk and never read.  1440B
                        # contiguous descriptors instead of 82B band runs.
                        sb = sbig_pool.tile(
                            [wb, hc * GW], BF16, tag=f"sb{kgrp}", name=f"sb{kgrp}_{b}_{ci}"
                        )
                        band = AP(
                            tensor=scr_ap.tensor,
                            offset=scr_ap.offset,
                            ap=[[ALPHA, wb], [1, hc * GW]],
                        )
                        nc.gpsimd.dma_start(sb[:], band)
                        sbig.append(sb)

                    # Transpose each row-block band to [d, w] and assemble.
                    abatch = asm_pool.tile([D, hc * W], F32, tag="abatch")
                    for hh in range(hc):
                        for kgrp, wb in enumerate(WGRPS):
                            t_ps = psum_t.tile([D, wb], BF16, tag="t_ps")
                            nc.tensor.matmul(
                                t_ps[:],
                                sbig[kgrp][:, hh * GW : hh * GW + D],
                                identity[0:wb, 0:wb],
                                start=True,
                                stop=True,
                                is_transpose=True,
                            )
                            dst = abatch[:, hh * W + 128 * kgrp : hh * W + 128 * kgrp + wb]
                            if kgrp < 2:
                                nc.vector.tensor_copy(dst, t_ps[:])
                            else:
                                nc.scalar.copy(dst, t_ps[:])

                    # SWDGE for the store: HWDGE pinned all 41 descriptors of
                    # this 41-partition DMA onto one SDMA engine (163us serial);
                    # gpsimd's CounterMachine sprays them across all 16.
                    nc.gpsimd.dma_start(
                        oute[b, :, h0 : h0 + hc, :],
                        abatch[:].rearrange("d (h w) -> d h w", w=W),
                    )

    nc.finalize()
    return nc


_compiled = {}


def _get_kernel(b_dim, hs):
    key = (b_dim, hs)
    if key not in _compiled:
        _compiled[key] = build_kernel(b_dim, hs)
    return _compiled[key]


def kernel(x_1: np.ndarray, x_2: np.ndarray) -> np.ndarray:
    assert x_1.shape == (B, C, H, W) and x_2.shape == (B, C, H, W)
    x_1 = np.ascontiguousarray(x_1, dtype=np.float32)
    x_2 = np.ascontiguousarray(x_2, dtype=np.float32)
    nc = _get_kernel(B, HS)
    in_maps = [
        {
            "x1": np.ascontiguousarray(x_1[:, :, i * HS : (i + 1) * HS, :]),
            "x2": np.ascontiguousarray(x_2[:, :, i * HS : (i + 1) * HS, :]),
        }
        for i in range(N_CORES)
    ]
    res = run_bass_kernel_spmd(nc, in_maps, core_ids=list(range(N_CORES))).results
    out = np.concatenate([res[i]["out"] for i in range(N_CORES)], axis=2)
    return out


# revision 10
# speedup vs baseline: 1.7235x; 1.0836x over previous
"""CorrelationLayer1D Trainium2 Bass kernel (v4).

Computes out[b, d, h, w] = sum_c x_1[b,c,h,w] * x2p[b,c,h,w+d] for d in [0, 41),
where x2p is x_2 width-padded by (8, 32).  Inputs [4,128,160,320] f32.

Sharding: data-parallel over H = 160 = 8*20 (correlation runs along W only, so
H-sharding needs no halo).  Per core, rows are processed in chunks of HC=10.

Structure per chunk (b, h-chunk):
 - inputs are cast f32->bf16 during the load DMA (SWDGE cast path on gpsimd).
 - PE computes per-row Grams in bf16 with M=32 stationary blocks, 4-way
   col-tiled via tile_position into [128|64, 72] PSUM tiles; partition p of a
   tile holds w-col 128*kgrp+p with its 41-wide band at cols (p%32)+d.
 - ScalarE copies each Gram tile into a bf16 atlas [wb, HC*72].
 - The per-partition band skew S[p,h,d] = G[p,h,(p%32)+d] is realized through
   a DRAM scratch round-trip (SBUF-side APs cannot shift per partition, DRAM
   APs can): the WRITE deskews - per 32-block bb the AP
   [[HC*72, 32],[72, HC],[1, 72]] at offset 32*(HC*72+1)*bb lands the band at
   the uniform address (HC*72+1)*p + 72*h + d - and the READ is then one plain
   2D full-plane AP [[HC*72+1, wb],[1, HC*72]] per group (1440B descriptors).
 - PE transposes each row's S [wb, 41] -> [41, wb] (bf16 identity matmul),
   VectorE copies (cast f32) into abatch [41, HC*320], one d-major store.

Engine streams form an explicit 2-deep software pipeline (engines execute
their streams in order, so cross-chunk work must interleave at emission):
  pool:   x1(k), x2(k) cast-loads + pad memsets, out-store(k-2)
  PE:     Gram matmuls(k), then transposes(k-1)
  scalar: PSUM->atlas copies(k)
  vector: transpose-PSUM->abatch copies(k-1)
  sync:   10 deskewing scratch writes(k), 3 full-plane reads(k)
The store runs on gpsimd/SWDGE because HWDGE pins all 41 descriptors of the
41-partition store onto a single SDMA engine (163us serial); SWDGE sprays
them across all 16.
"""

import sys

import numpy as np

try:
    import concourse.bass as bass  # noqa: F401
except ImportError:
    sys.path.insert(0, "/opt/trn_rl_repo")

import concourse.bass as bass
import concourse.tile as tile
from concourse import bacc, masks, mybir
from concourse.ap import AP
from concourse.bass_utils import run_bass_kernel_spmd

MAX_DISP = 40
D = MAX_DISP + 1  # 41 displacements
PAD_L = 8
PAD_R = 32
B, C, H, W = 4, 128, 160, 320
N_CORES = 8
HS = H // N_CORES  # 20 h-rows per core
WP = W + PAD_L + PAD_R  # 360
WGRPS = [128, 128, 64]  # partition-group widths covering W=320
GW = 32 + MAX_DISP  # compacted gram width per 32-col block: 72

F32 = mybir.dt.float32
BF16 = mybir.dt.bfloat16


def build_kernel(b_dim=B, hs=HS, hc=None):
    if hc is None:
        hc = 10 if hs % 10 == 0 else hs
    assert hs % hc == 0
    nchunks = hs // hc
    ALPHA = hc * GW + 1

    nc = bacc.Bacc("TRN2", target_bir_lowering=False, debug=False)
    x1e = nc.declare_dram_parameter("x1", [b_dim, C, hs, W], F32, isOutput=False)
    x2e = nc.declare_dram_parameter("x2", [b_dim, C, hs, W], F32, isOutput=False)
    oute = nc.declare_dram_parameter("out", [b_dim, D, hs, W], F32, isOutput=True)

    with tile.TileContext(nc) as tc:
        with (
            tc.tile_pool(name="const", bufs=1) as const_pool,
            tc.tile_pool(name="xin", bufs=3) as xin_pool,
            tc.tile_pool(name="atlas", bufs=2) as atlas_pool,
            tc.tile_pool(name="sbig", bufs=3) as sbig_pool,
            tc.tile_pool(name="asm", bufs=3) as asm_pool,
            tc.tile_pool(name="psum_g", bufs=5, space="PSUM") as psum_g,
            tc.tile_pool(name="psum_t", bufs=3, space="PSUM") as psum_t,
            tc.tile_pool(name="scratch", bufs=3, space="DRAM") as scratch_pool,
        ):
            identity = const_pool.tile([128, 128], BF16)
            masks.make_identity(nc, identity[:])

            def emit_loads(b, ci, k):
                h0 = ci * hc
                x1b = xin_pool.tile([C, hc * W], BF16, tag="x1b", name=f"x1b_{k}")
                nc.gpsimd.dma_start(
                    x1b[:].rearrange("p (h w) -> p h w", w=W),
                    x1e[b, :, h0 : h0 + hc, :],
                )
                x2b = xin_pool.tile([C, hc * WP], BF16, tag="x2b", name=f"x2b_{k}")
                x2b3 = x2b[:].rearrange("p (h w) -> p h w", w=WP)
                nc.gpsimd.memset(x2b3[:, :, 0:PAD_L], 0.0)
                nc.gpsimd.memset(x2b3[:, :, PAD_L + W : WP], 0.0)
                nc.gpsimd.dma_start(
                    x2b3[:, :, PAD_L : PAD_L + W], x2e[b, :, h0 : h0 + hc, :]
                )
                return x1b, x2b

            def emit_grams(x1b, x2b, k):
                atlas = [
                    atlas_pool.tile([wb, hc * GW], BF16, tag=f"atl{g}", name=f"atl{g}_{k}")
                    for g, wb in enumerate(WGRPS)
                ]
                for hh in range(hc):
                    o1 = hh * W
                    o2 = hh * WP
                    for kgrp, wb in enumerate(WGRPS):
                        gram_ps = psum_g.tile(
                            [wb, GW], F32, tag="gram", name=f"gram_{k}_{hh}_{kgrp}"
                        )
                        for kk in range(wb // 32):
                            w0 = 128 * kgrp + 32 * kk
                            nc.tensor.matmul(
                                gram_ps[32 * kk : 32 * kk + 32, :],
                                x1b[:, o1 + w0 : o1 + w0 + 32],
                                x2b[:, o2 + w0 : o2 + w0 + GW],
                                start=True,
                                stop=True,
                                tile_position=(0, 32 * kk),
                            )
                        nc.scalar.copy(
                            atlas[kgrp][:, hh * GW : (hh + 1) * GW], gram_ps[:]
                        )
                return atlas

            def emit_roundtrip(atlas, k):
                sbig = []
                for kgrp, wb in enumerate(WGRPS):
                    scr = scratch_pool.tile(
                        [wb * ALPHA], BF16, tag=f"scr{kgrp}", name=f"scr{kgrp}_{k}"
                    )
                    scr_ap = scr[:]
                    for bb in range(wb // 32):
                        dst = AP(
                            tensor=scr_ap.tensor,
                            offset=scr_ap.offset + 32 * ALPHA * bb,
                            ap=[[hc * GW, 32], [GW, hc], [1, GW]],
                        )
                        nc.sync.dma_start(dst, atlas[kgrp][32 * bb : 32 * bb + 32, :])
                    sb = sbig_pool.tile(
                        [wb, hc * GW], BF16, tag=f"sb{kgrp}", name=f"sb{kgrp}_{k}"
                    )
                    band = AP(
                        tensor=scr_ap.tensor,
                        offset=scr_ap.offset,
                        ap=[[ALPHA, wb], [1, hc * GW]],
                    )
                    nc.sync.dma_start(sb[:], band)
                    sbig.append(sb)
                return sbig

            def emit_transposes(sbig, k):
                abatch = asm_pool.tile([D, hc * W], F32, tag="abatch", name=f"ab_{k}")
                for hh in range(hc):
                    for kgrp, wb in enumerate(WGRPS):
                        t_ps = psum_t.tile(
                            [D, wb], BF16, tag="t_ps", name=f"t_{k}_{hh}_{kgrp}"
                        )
                        nc.tensor.matmul(
                            t_ps[:],
                            sbig[kgrp][:, hh * GW : hh * GW + D],
                            identity[0:wb, 0:wb],
                            start=True,
                            stop=True,
                            is_transpose=True,
                        )
                        nc.vector.tensor_copy(
                            abatch[:, hh * W + 128 * kgrp : hh * W + 128 * kgrp + wb],
                            t_ps[:],
                        )
                return abatch

            def emit_out(st):
                b, ci, abatch = st["b"], st["ci"], st["abatch"]
                h0 = ci * hc
                nc.gpsimd.dma_start(
                    oute[b, :, h0 : h0 + hc, :],
                    abatch[:].rearrange("d (h w) -> d h w", w=W),
                )

            seq = [(b, ci) for b in range(b_dim) for ci in range(nchunks)]
            pend = {}
            for k, (b, ci) in enumerate(seq):
                x1b, x2b = emit_loads(b, ci, k)
                if k >= 2:
                    emit_out(pend.pop(k - 2))
                atlas = emit_grams(x1b, x2b, k)
                if k >= 1:
                    pend[k - 1]["abatch"] = emit_transposes(pend[k - 1]["sbig"], k - 1)
                sbig = emit_roundtrip(atlas, k)
                pend[k] = {"b": b, "ci": ci, "sbig": sbig}
            # drain
            last = len(seq) - 1
            pend[last]["abatch"] = emit_transposes(pend[last]["sbig"], last)
            if last >= 1:
                emit_out(pend.pop(last - 1))
            emit_out(pend.pop(last))

    nc.finalize()
    return nc


_compiled = {}


def _get_kernel(b_dim, hs):
    key = (b_dim, hs)
    if key not in _compiled:
        _compiled[key] = build_kernel(b_dim, hs)
    return _compiled[key]


def kernel(x_1: np.ndarray, x_2: np.ndarray) -> np.ndarray:
    assert x_1.shape == (B, C, H, W) and x_2.shape == (B, C, H, W)
    x_1 = np.ascontiguousarray(x_1, dtype=np.float32)
    x_2 = np.ascontiguousarray(x_2, dtype=np.float32)
    nc = _get_kernel(B, HS)
    in_maps = [
        {
            "x1": np.ascontiguousarray(x_1[:, :, i * HS : (i + 1) * HS, :]),
            "x2": np.ascontiguousarray(x_2[:, :, i * HS : (i + 1) * HS, :]),
        }
        for i in range(N_CORES)
    ]
    res = run_bass_kernel_spmd(nc, in_maps, core_ids=list(range(N_CORES))).results
    out = np.concatenate([res[i]["out"] for i in range(N_CORES)], axis=2)
    return out
